# revision 48
# baseline (speedup 1.0000x reference)
"""Trainium2 Bass kernel for nn_MultiModalFusionNetwork_88759794139604.

6-layer transformer encoder with relative-position attention bias.
Data-parallel over batch across 8 NeuronCores (2 batch elements/core),
bf16 matmuls with fp32 accumulation/softmax/layernorm.

kernel(**inputs) takes the FULL unsharded inputs (as produced by
setup_inputs()) and returns the FULL [16, 512, 1] float32 output.
"""
import numpy as np
import jax

# Bass/Tile kernel for nn_MultiModalFusionNetwork: 6-layer transformer encoder
# with relative-position attention bias, data-parallel over batch on 8 cores.
import os
import numpy as np
import ml_dtypes

import concourse.bass as bass
import concourse.bacc as bacc
import concourse.mybir as mybir
import concourse.tile as tile

F32 = mybir.dt.float32
BF16 = mybir.dt.bfloat16
AF = mybir.ActivationFunctionType
ALU = mybir.AluOpType

L, D, H, FF, I, MAXREL = 6, 512, 8, 2048, 50, 128
HD = D // H          # 64
NREL = 2 * MAXREL + 1  # 257
EPS = 1e-5
S = 512              # seq len
BPC = 2              # batch per core
T = BPC * S          # tokens per core = 1024
NT = T // 128        # 8 token tiles
NC = D // 128        # 4 feature chunks
NQC = S // 128       # 4 q-chunks per sequence
NFC = FF // 128      # 16
BAND = 2 * MAXREL + 255  # 511: dram row width [pad127 | 257 | pad127]

bf16c = lambda a: np.ascontiguousarray(np.asarray(a).astype(ml_dtypes.bfloat16))
f32c = lambda a: np.ascontiguousarray(np.asarray(a).astype(np.float32))


def _sinusoidal_pe(seq_len, d_model):
    pos = np.arange(seq_len, dtype=np.float32)[:, None]
    div = np.exp(np.arange(0, d_model, 2, dtype=np.float32) * (-np.log(10000.0) / d_model))
    pe = np.zeros((seq_len, d_model), np.float32)
    pe[:, 0::2] = np.sin(pos * div)
    pe[:, 1::2] = np.cos(pos * div)
    return pe


# Build-time specialization flags, set by prep_host from the actual input
# values (build_nc reads them; kernel() caches per flag tuple).
_FLAGS = {"trivial_ln": False, "trivial_bias": False}


def prep_host(p):
    """inputs (numpy dict) -> (shared weight arrays, per-core xT list)"""
    p = {k: np.asarray(v) for k, v in p.items()}
    _FLAGS["trivial_ln"] = bool(
        np.all(p['ln1_g'] == 1) and np.all(p['ln1_b'] == 0)
        and np.all(p['ln2_g'] == 1) and np.all(p['ln2_b'] == 0)
        and np.all(p['on_g'] == 1) and np.all(p['on_b'] == 0))
    _FLAGS["trivial_bias"] = bool(
        np.all(p['bo'] == 0) and np.all(p['b2'] == 0))
    pe = _sinusoidal_pe(S, D)
    bias_pe = pe + p['in_b'][None, :]
    bias_pe2 = np.concatenate([bias_pe, bias_pe], axis=0)   # [T, D]

    # rel_rhs [L, 128, 259]: cols 0..256 rel_emb[l].T (dup both halves),
    # col 257 = rel_emb[l,0] (cL), col 258 = rel_emb[l,NREL-1] (cR)
    rel_rhs = np.zeros((L, 128, NREL + 2), np.float32)
    for half in range(2):
        sl = slice(half * HD, (half + 1) * HD)
        rel_rhs[:, sl, :NREL] = np.transpose(p['rel_emb'], (0, 2, 1))
        rel_rhs[:, sl, NREL] = p['rel_emb'][:, 0, :]
        rel_rhs[:, sl, NREL + 1] = p['rel_emb'][:, NREL - 1, :]

    shared = {
        'in_w': bf16c(p['in_w']),
        'bias_pe': f32c(bias_pe2),
        'Wqkvo': bf16c(np.concatenate(
            [p['Wq'], p['Wk'] * (1.0 / np.sqrt(HD)), p['Wv'], p['Wo']], axis=2)),
        'rel_rhs': bf16c(rel_rhs),
        'w1': bf16c(p['w1']),
        'b1p': f32c(np.transpose(p['b1'].reshape(L, NFC, 128), (0, 2, 1))),
        'w2': bf16c(p['w2']),
        'ln6': f32c(np.concatenate(
            [p['bo'], p['b2'], p['ln1_g'], p['ln1_b'], p['ln2_g'], p['ln2_b']],
            axis=1)),
        'on_g': f32c(p['on_g']), 'on_b': f32c(p['on_b']),
        'p1_w': bf16c(p['p1_w']),
        'p1bp': f32c(p['p1_b'].reshape(2, 128).T),          # [128, 2]
        'p2_w': bf16c(p['p2_w']),
        'p2_b': f32c(p['p2_b']),
        'ident': bf16c(np.eye(128, dtype=np.float32)),
    }
    x = p['x']
    xs = [bf16c(x[c * BPC:(c + 1) * BPC].reshape(T, I).T) for c in range(8)]
    return shared, xs


def build_nc(num_cores=8, skip_attn=False, skip_ffn=False, skip_rel=False):
    import os
    skip_attn = skip_attn or os.environ.get("SKIP_ATTN") == "1"
    skip_ffn = skip_ffn or os.environ.get("SKIP_FFN") == "1"
    skip_rel = skip_rel or os.environ.get("SKIP_REL") == "1"
    repeat = int(os.environ.get("REPEAT", 1))
    triv_ln = _FLAGS["trivial_ln"]
    triv_bias = _FLAGS["trivial_bias"]

    nc = bacc.Bacc("TRN2", target_bir_lowering=False, debug=False,
                   num_devices=num_cores)

    dT = lambda n, s, dt: nc.dram_tensor(n, s, dt, kind="ExternalInput")
    dx = dT("xT", [I, T], BF16)
    din_w = dT("in_w", [I, D], BF16)
    dbias_pe = dT("bias_pe", [T, D], F32)
    dWqkvo = dT("Wqkvo", [L, D, 4 * D], BF16)
    drel = dT("rel_rhs", [L, 128, NREL + 2], BF16)
    dw1 = dT("w1", [L, D, FF], BF16)
    db1p = dT("b1p", [L, 128, NFC], F32)
    dw2 = dT("w2", [L, FF, D], BF16)
    dln6 = dT("ln6", [L, 6 * D], F32)
    don_g = dT("on_g", [D], F32)
    don_b = dT("on_b", [D], F32)
    dp1w = dT("p1_w", [D, D // 2], BF16)
    dp1bp = dT("p1bp", [128, 2], F32)
    dp2w = dT("p2_w", [D // 2, 1], BF16)
    dp2b = dT("p2_b", [1], F32)
    dident = dT("ident", [128, 128], BF16)
    dy = nc.dram_tensor("y", [T], F32, kind="ExternalOutput")

    with tile.TileContext(nc) as tc:
        # ---------------- pools ----------------
        import contextlib
        es = contextlib.ExitStack()
        with es:
            P = lambda name, bufs, **kw: es.enter_context(
                tc.tile_pool(name=name, bufs=bufs, **kw))
            constp = P("const", 1)

            def cst(shape, dt, nm):
                return constp.tile(shape, dt, tag=nm, name=nm)

            ident = cst([128, 128], BF16, "ident")
            nc.sync.dma_start(out=ident[:], in_=dident[:, :])
            ones_row = cst([1, 128], BF16, "ones_row")
            nc.vector.memset(ones_row[:], 1.0)
            epst = cst([128, 1], F32, "epst")
            nc.vector.memset(epst[:], EPS)
            xT = cst([I, T], BF16, "xT")
            nc.sync.dma_start(out=xT[:], in_=dx[:, :])
            inw = cst([I, D], BF16, "inw")
            nc.sync.dma_start(out=inw[:], in_=din_w[:, :])
            p1w = cst([128, NC, D // 2], BF16, "p1w")
            nc.sync.dma_start(out=p1w[:], in_=dp1w.rearrange("(c p) n -> p c n", p=128))
            p1b = cst([128, 2], F32, "p1b")
            nc.sync.dma_start(out=p1b[:], in_=dp1bp[:, :])
            p2w = cst([128, 2], BF16, "p2w")
            nc.sync.dma_start(out=p2w[:], in_=dp2w.rearrange("(c p) n -> p c n", p=128)[:, :, 0])
            p2b = cst([1, 1], F32, "p2b")
            nc.sync.dma_start(out=p2b[:], in_=dp2b[None, :])
            hpool = P("h", int(os.environ.get("B_H", 10)))       # fp32 residual [128,512]
            rpool = P("r", 4)        # fp32 pre-LN [128,512]
            htp = P("hT", 6)         # bf16 [128,1024] feature-major
            hcp = P("hc", 4)         # bf16 cast tmp [128,512]
            qkp = P("qk", 8)         # qT/kT bf16 [128,1024] (tags qt/kt, bufs 4)
            vp = P("v", int(os.environ.get("B_V", 8)))           # v bf16 [128,512]
            ctxp = P("ctxT", int(os.environ.get("B_CTX", 8)))      # bf16 [128,512]
            attp = P("att", int(os.environ.get("B_ATT", 10)))       # attn bf16 [128,512]
            attTp = P("attT", int(os.environ.get("B_ATTT", 4)))     # attnT bf16 [128,512]
            qrlp = P("qrl", int(os.environ.get("B_QRL", 3)))       # qrel row bf16 [128,511]
            bndp = P("bnd", int(os.environ.get("B_BND", 3)))       # band diag bf16 [128,384]
            smlp = P("sml", 8)       # small fp32 stats
            wqp = P("wq", 2)
            wkp = P("wk", 2)
            wvp = P("wv", 2)
            wop = P("wo", 2)
            relp = P("relw", 2)
            w1p = P("w1", 2)
            w2p = P("w2", 2)
            lnp = P("lnt", 2)        # per-layer bcast consts (distinct tags)
            a1p = P("a1", 4)         # ffn act bf16 [128,1024]
            drp = P("dr", int(os.environ.get("B_DR", 3)), space="DRAM")

            def bcast(drow, tag, width=D):
                t = lnp.tile([128, width], F32, tag=tag, name=tag)
                ap = bass.AP(tensor=drow.tensor, offset=drow.offset,
                             ap=[[0, 128]] + [list(x) for x in drow.ap])
                nc.sync.dma_start(out=t[:], in_=ap)
                return t

            # ---------------- input projection ----------------
            # (repeat loop for on-hw timing: recomputes the same network)
            for _rep in range(repeat):
              h = []
              with tc.tile_pool(name="ps_init", bufs=4, space="PSUM") as psi, \
                   tc.tile_pool(name="bpe", bufs=2) as bpep:
                  for tt in range(NT):
                      bp = bpep.tile([128, D], F32, name="bp")
                      nc.sync.dma_start(out=bp[:], in_=dbias_pe[tt * 128:(tt + 1) * 128, :])
                      ps = psi.tile([128, D], F32, name="ps")
                      nc.tensor.matmul(ps[:], xT[:, tt * 128:(tt + 1) * 128], inw[:])
                      ht = hpool.tile([128, D], F32, tag="h", name="ht")
                      nc.vector.tensor_add(ht[:], ps[:], bp[:])
                      h.append(ht)

              # ---------------- helpers ----------------
              def transpose_to_fmajor(src, psum_pool):
                  pss = [psum_pool.tile([128, T], BF16, name="tps") for _ in range(NC)]
                  for tt in range(NT):
                      c = hcp.tile([128, D], BF16, name="hcst")
                      nc.vector.tensor_copy(c[:], src[tt][:])
                      for cc in range(NC):
                          nc.tensor.transpose(pss[cc][:, tt * 128:(tt + 1) * 128],
                                              c[:, cc * 128:(cc + 1) * 128],
                                              ident[:])
                  out = []
                  for cc in range(NC):
                      t = htp.tile([128, T], BF16, name="hTt")
                      nc.vector.tensor_copy(t[:], pss[cc][:])
                      out.append(t)
                  return out

              def layer_norm(r, g_t, b_t):
                  st = smlp.tile([128, 6], F32, tag="st", name="st")
                  nc.vector.bn_stats(out=st[:], in_=r[:])
                  mv = smlp.tile([128, 2], F32, tag="mv", name="mv")
                  nc.vector.bn_aggr(out=mv[:], in_=st[:])
                  sd = smlp.tile([128, 1], F32, tag="sd", name="sd")
                  nc.scalar.activation(sd[:], mv[:, 1:2], AF.Sqrt, bias=epst[:])
                  rs = smlp.tile([128, 1], F32, tag="rs", name="rs")
                  nc.vector.reciprocal(rs[:], sd[:])
                  nrm = hpool.tile([128, D], F32, tag="h", name="nrm")
                  nc.vector.tensor_scalar(nrm[:], r[:], mv[:, 0:1], rs[:],
                                          op0=ALU.subtract, op1=ALU.mult)
                  if g_t is not None:
                      nc.gpsimd.tensor_mul(nrm[:], nrm[:], g_t[:])
                      nc.gpsimd.tensor_add(nrm[:], nrm[:], b_t[:])
                  return nrm

              # ---------------- layers ----------------
              def load_layer_weights(l):
                  wall = wqp.tile([128, NC, 4, D], BF16, name="wall")
                  nc.sync.dma_start(
                      out=wall[:],
                      in_=dWqkvo[l].rearrange("(c p) (w n) -> p c w n", p=128, w=4))
                  rel = relp.tile([128, NREL + 2], BF16, name="relt")
                  nc.sync.dma_start(out=rel[:], in_=drel[l])
                  b1 = lnp.tile([128, NFC], F32, tag="b1", name="b1")
                  nc.sync.dma_start(out=b1[:], in_=db1p[l])
                  return wall, rel, b1

              nxt_w = load_layer_weights(0)
              for l in range(L):
                  wall, rel, b1 = nxt_w
                  wq, wk, wv, wo = (wall[:, :, 0, :], wall[:, :, 1, :],
                                    wall[:, :, 2, :], wall[:, :, 3, :])
                  if triv_ln and triv_bias:
                      bo_t = b2_t = g1_t = be1_t = g2_t = be2_t = None
                  else:
                      ln6 = bcast(dln6[l], "ln6", width=6 * D)
                      bo_t, b2_t = ln6[:, 0:D], ln6[:, D:2 * D]
                      g1_t, be1_t = ln6[:, 2 * D:3 * D], ln6[:, 3 * D:4 * D]
                      g2_t, be2_t = ln6[:, 4 * D:5 * D], ln6[:, 5 * D:6 * D]

                  with tc.tile_pool(name="ps_t", bufs=4, space="PSUM") as pst:
                      hT = transpose_to_fmajor(h, pst)

                  # -- QKV --
                  qT, kT, v = [], [], []
                  with tc.tile_pool(name="ps_qk", bufs=3, space="PSUM") as psqk, \
                       tc.tile_pool(name="ps_v", bufs=2, space="PSUM") as psv:
                      for cc in range(NC):
                          for w, dst, tg in ((wq, qT, "qt"), (wk, kT, "kt")):
                              ps = psqk.tile([128, T], F32, name="qkps")
                              for kc in range(NC):
                                  for hf in range(2):
                                      nc.tensor.matmul(
                                          ps[:, hf * 512:(hf + 1) * 512],
                                          w[:, kc, cc * 128:(cc + 1) * 128],
                                          hT[kc][:, hf * 512:(hf + 1) * 512],
                                          start=(kc == 0), stop=(kc == NC - 1))

                              t = qkp.tile([128, T], BF16, tag=tg, bufs=4, name=tg)
                              nc.scalar.copy(t[:], ps[:])
                              dst.append(t)
                      for tt in range(NT):
                          ps = psv.tile([128, H, HD], F32, name="vps")
                          psf = bass.AP(tensor=ps.tensor, offset=ps.offset,
                                        ap=[[D, 128], [1, D]])
                          for kc in range(NC):
                              nc.tensor.matmul(psf, hT[kc][:, tt * 128:(tt + 1) * 128],
                                               wv[:, kc, :], start=(kc == 0), stop=(kc == NC - 1))
                          # v_aug: per head gh a 65-col block [v, 1]; the ones
                          # column folds the softmax denominator into the ctx
                          # matmul as an extra output row.
                          t = vp.tile([128, H, 65], BF16, name="vt")
                          nc.scalar.copy(t[:, :, 0:HD], ps[:, :, :])
                          nc.vector.memset(t[:, :, HD:65], 1.0)
                          v.append(t)

                  # prefetch next layer's weights during attention
                  if l + 1 < L:
                      nxt_w = load_layer_weights(l + 1)

                  # -- attention: k-major (transposed) scores --
                  # scoresT[k,q] = k·q + band; exp in place; ctxT = v_augᵀ·expT
                  # (ones col of v_aug yields the softmax denominator row);
                  # per-q normalize via a DRAM-bounced reciprocal broadcast.
                  ctxT = [[None] * NC for _ in range(BPC)]
                  assert not skip_attn
                  # near-band / clamped-far column ranges per k-chunk
                  NEARC = {0: (0, 256), 1: (0, 384), 2: (128, 512), 3: (256, 512)}
                  FARC = {0: (0, 256, 512), 1: (0, 384, 512),
                          2: (1, 0, 128), 3: (1, 0, 256)}  # (cLR row, a, b)
                  with tc.tile_pool(name="ps_sc", bufs=int(os.environ.get("B_SC", 3)), space="PSUM") as pssc, \
                       tc.tile_pool(name="ps_qr", bufs=int(os.environ.get("B_QR", 2)), space="PSUM") as psqr, \
                       tc.tile_pool(name="ps_cx", bufs=int(os.environ.get("B_CX", 2)), space="PSUM") as pscx:
                      QSLAB = 128 * BAND  # per-qc slab in the qdr scratch

                      def rel_produce(bb, cch):
                          out = {}
                          qdrs, qsbs = {}, {}
                          for hh in range(2):
                              qdrs[hh] = drp.tile([S * BAND], BF16, name="qdr")
                              qsbs[hh] = qrlp.tile([128, NQC, BAND], BF16, name="qsb")
                          for qc in range(NQC):
                              qsl = slice(bb * S + qc * 128, bb * S + (qc + 1) * 128)
                              qpss = {}
                              for hh in range(2):
                                  rows = slice(hh * 64, (hh + 1) * 64)
                                  qps = psqr.tile([128, D], F32, name="qps")
                                  nc.tensor.matmul(qps[:, 0:NREL + 2],
                                                   qT[cch][rows, qsl], rel[rows, :])
                                  qpss[hh] = qps
                              for hh in range(2):
                                  qps, qsb = qpss[hh], qsbs[hh]
                                  if qc % 2 == 0:
                                      nc.scalar.copy(qsb[:, qc, 127:127 + NREL],
                                                     qps[:, 0:NREL])
                                  else:
                                      nc.vector.tensor_copy(qsb[:, qc, 127:127 + NREL],
                                                            qps[:, 0:NREL])
                                  clr = smlp.tile([128, 2], F32, tag="clr", name="clr")
                                  nc.vector.tensor_copy(clr[:], qps[:, NREL:NREL + 2])
                                  nc.gpsimd.tensor_scalar(
                                      qsb[:, qc, 0:127], qsb[:, qc, 127:254], 0.0,
                                      clr[:, 0:1], op0=ALU.mult, op1=ALU.add)
                                  nc.gpsimd.tensor_scalar(
                                      qsb[:, qc, 127 + NREL:BAND], qsb[:, qc, 127:254],
                                      0.0, clr[:, 1:2], op0=ALU.mult, op1=ALU.add)
                          for hh in range(2):
                              rows = slice(hh * 64, (hh + 1) * 64)
                              # clamped-far row values cL/cR for every q: [2, S]
                              cLR = bndp.tile([1, 2 * S], BF16, tag="cLR", name="cLR")
                              for rr in range(2):
                                  clps = psqr.tile([1, S], F32, tag="clps", bufs=1,
                                                   name="clps")
                                  nc.tensor.matmul(
                                      clps[:], rel[rows, NREL + rr:NREL + rr + 1],
                                      qT[cch][rows, bb * S:(bb + 1) * S])
                                  nc.scalar.copy(cLR[0:1, rr * S:(rr + 1) * S],
                                                 clps[:])
                              qdr, qsb = qdrs[hh], qsbs[hh]
                              nc.scalar.dma_start(
                                  out=bass.AP(tensor=qdr.tensor, offset=qdr.offset,
                                              ap=[[BAND, 128], [QSLAB, NQC], [1, BAND]]),
                                  in_=qsb[:])
                              # q-major skewed gathers (fast: contiguous inner
                              # k); transposed into the score psum later by PE
                              bt0 = bndp.tile([128, 256], BF16, tag="bt0", name="bt0")
                              nc.sync.dma_start(
                                  out=bt0[:],
                                  in_=bass.AP(tensor=qdr.tensor,
                                              offset=qdr.offset + 255,
                                              ap=[[BAND - 1, 128], [1, 256]]))
                              bt12 = bndp.tile([128, 2, 384], BF16, tag="bt12",
                                               name="bt12")
                              nc.sync.dma_start(
                                  out=bt12[:],
                                  in_=bass.AP(tensor=qdr.tensor,
                                              offset=qdr.offset + QSLAB + 127,
                                              ap=[[BAND - 1, 128], [QSLAB, 2],
                                                  [1, 384]]))
                              bt3 = bndp.tile([128, 256], BF16, tag="bt3", name="bt3")
                              nc.sync.dma_start(
                                  out=bt3[:],
                                  in_=bass.AP(tensor=qdr.tensor,
                                              offset=qdr.offset + 3 * QSLAB + 127,
                                              ap=[[BAND - 1, 128], [1, 256]]))
                              out[hh] = ((bt0, bt12, bt3), cLR)
                          return out

                      def attn_core(bb, cch, rp):
                          cps = {}
                          for hh in range(2):
                              cps[hh] = pscx.tile([128, S], F32, name="cxps")
                          for kc in range(NQC):
                              ee = {}
                              for hh in range(2):
                                  rows = slice(hh * 64, (hh + 1) * 64)
                                  bts, cLR = rp[hh]
                                  bt0, bt12, bt3 = bts
                                  sps = pssc.tile([128, S], F32, name="scps")
                                  nc.tensor.matmul(
                                      sps[:],
                                      kT[cch][rows, bb * S + kc * 128:
                                              bb * S + (kc + 1) * 128],
                                      qT[cch][rows, bb * S:(bb + 1) * S],
                                      start=True, stop=False)
                                  # band add: PE-transpose q-major bt blocks
                                  a, b = NEARC[kc]
                                  for qc in range(a // 128, b // 128):
                                      if qc == 0:
                                          blk = bt0[:, kc * 128:(kc + 1) * 128]
                                      elif qc == 1:
                                          blk = bt12[:, 0, kc * 128:(kc + 1) * 128]
                                      elif qc == 2:
                                          blk = bt12[:, 1, (kc - 1) * 128:kc * 128]
                                      else:
                                          blk = bt3[:, (kc - 2) * 128:(kc - 1) * 128]
                                      nc.tensor.matmul(
                                          sps[:, qc * 128:(qc + 1) * 128],
                                          blk, ident[:], start=False, stop=False)
                                  rr, fa, fb = FARC[kc]
                                  nc.tensor.matmul(sps[:, fa:fb], ones_row[:],
                                                   cLR[0:1, rr * S + fa:rr * S + fb],
                                                   start=False, stop=True)
                                  et = attp.tile([128, S], BF16, tag="exp", name="expt")
                                  nc.scalar.activation(et[:], sps[:], AF.Exp)
                                  ee[hh] = et
                              for hh in range(2):
                                  gh = 2 * cch + hh
                                  nc.tensor.matmul(
                                      cps[hh][0:65, :],
                                      v[bb * NQC + kc][:, gh, :],
                                      ee[hh][:],
                                      start=(kc == 0), stop=(kc == NQC - 1))
                          # denominators sit at row 64 of each cps; bounce
                          # through DRAM to broadcast, reciprocal done wide
                          den = rpool.tile([128, S], F32, tag="dn", bufs=2, name="den")
                          nc.scalar.copy(den[64:65, :], cps[0][64:65, :])
                          nc.scalar.copy(den[0:1, :], cps[1][64:65, :])
                          drc = drp.tile([2 * S], F32, tag="drc", name="drc")
                          nc.sync.dma_start(
                              out=bass.AP(tensor=drc.tensor, offset=drc.offset,
                                          ap=[[1, S]]),
                              in_=den[64:65, :])
                          nc.sync.dma_start(
                              out=bass.AP(tensor=drc.tensor, offset=drc.offset + S,
                                          ap=[[1, S]]),
                              in_=den[0:1, :])
                          rcb0 = rpool.tile([64, S], F32, tag="rcb0", bufs=2,
                                            name="rcb0")
                          rcb1 = rpool.tile([64, S], F32, tag="rcb1", bufs=2,
                                            name="rcb1")
                          nc.sync.dma_start(
                              out=rcb0[0:64, :],
                              in_=bass.AP(tensor=drc.tensor, offset=drc.offset,
                                          ap=[[0, 64], [1, S]]))
                          nc.sync.dma_start(
                              out=rcb1[0:64, :],
                              in_=bass.AP(tensor=drc.tensor, offset=drc.offset + S,
                                          ap=[[0, 64], [1, S]]))
                          nc.vector.reciprocal_approx_fast(out=rcb0[:], in_=rcb0[:])
                          nc.vector.reciprocal_approx_fast(out=rcb1[:], in_=rcb1[:])
                          ct = ctxp.tile([128, S], BF16, name="ct")
                          nc.vector.tensor_mul(ct[0:64, :], cps[0][0:64, :],
                                               rcb0[:])
                          nc.vector.tensor_mul(ct[64:128, :], cps[1][0:64, :],
                                               rcb1[:])
                          ctxT[bb][cch] = ct

                      pairs = [(bb, hp2) for bb in range(BPC) for hp2 in range(H // 2)]
                      DEPTH = int(os.environ.get("PIPE_DEPTH", 2))
                      rps = {}
                      for i in range(min(DEPTH, len(pairs))):
                          rps[i] = rel_produce(*pairs[i])
                      for i, pr in enumerate(pairs):
                          if i + DEPTH < len(pairs):
                              rps[i + DEPTH] = rel_produce(*pairs[i + DEPTH])
                          attn_core(*pr, rps.pop(i))


                  # -- Wo + residual + LN1 --
                  h2 = []
                  with tc.tile_pool(name="ps_wo", bufs=4, space="PSUM") as pswo:
                      for tt in range(NT):
                          b, tq = tt // NQC, tt % NQC
                          ps = pswo.tile([128, D], F32, name="wops")
                          for cc in range(NC):
                              nc.tensor.matmul(ps[:], ctxT[b][cc][:, tq * 128:(tq + 1) * 128],
                                               wo[:, cc, :], start=(cc == 0), stop=(cc == NC - 1))
                          r = rpool.tile([128, D], F32, tag="r", name="r")
                          nc.vector.tensor_add(r[:], ps[:], h[tt][:])
                          if bo_t is not None:
                              nc.gpsimd.tensor_add(r[:], r[:], bo_t[:])
                          h2.append(layer_norm(r, g1_t, be1_t))

                  # -- FFN (stream a1 per ff-chunk, half the tokens at a time) --
                  h3 = []
                  if skip_ffn:
                      h = h2
                      continue
                  with tc.tile_pool(name="ps_t2", bufs=4, space="PSUM") as pst2:
                      hT2 = transpose_to_fmajor(h2, pst2)
                  if True:
                      with tc.tile_pool(name="ps_f1", bufs=2, space="PSUM") as psf1, \
                           tc.tile_pool(name="ps_f2", bufs=4, space="PSUM") as psf2:
                          for hf in range(2):
                              f2ps = [psf2.tile([128, D], F32, name="f2ps")
                                      for _ in range(4)]
                              for fg in range(4):  # groups of 4 fc
                                  w1c = w1p.tile([128, NC, 512], BF16, name="w1c")
                                  nc.sync.dma_start(
                                      out=w1c[:],
                                      in_=dw1[l][:, fg * 512:(fg + 1) * 512]
                                          .rearrange("(c p) m -> p c m", p=128))
                                  w2c = w2p.tile([128, 4, D], BF16, name="w2c")
                                  nc.sync.dma_start(
                                      out=w2c[:],
                                      in_=dw2[l][fg * 512:(fg + 1) * 512, :]
                                          .rearrange("(f p) n -> p f n", p=128))
                                  for fi in range(4):
                                      fc = fg * 4 + fi
                                      ps = psf1.tile([128, 512], F32, name="f1ps")
                                      for kc in range(NC):
                                          nc.tensor.matmul(ps[:],
                                                           w1c[:, kc, fi * 128:(fi + 1) * 128],
                                                           hT2[kc][:, hf * 512:(hf + 1) * 512],
                                                           start=(kc == 0), stop=(kc == NC - 1))
                                      t = a1p.tile([128, 512], BF16, tag="a1", name="a1t")
                                      nc.scalar.activation(t[:], ps[:], AF.Relu,
                                                           bias=b1[:, fc:fc + 1])
                                      for tq in range(4):
                                          nc.tensor.matmul(f2ps[tq][:],
                                                           t[:, tq * 128:(tq + 1) * 128],
                                                           w2c[:, fi, :], start=(fc == 0),
                                                           stop=(fc == NFC - 1))
                              for tq in range(4):
                                  tt = hf * 4 + tq
                                  r = rpool.tile([128, D], F32, tag="r", name="r2")
                                  nc.vector.tensor_add(r[:], f2ps[tq][:], h2[tt][:])
                                  if b2_t is not None:
                                      nc.gpsimd.tensor_add(r[:], r[:], b2_t[:])
                                  h3.append(layer_norm(r, g2_t, be2_t))
                  h = h3

              # ---------------- final LN + head ----------------
              if triv_ln:
                  ong_t = onb_t = None
              else:
                  ong_t = bcast(don_g[:], "ong")
                  onb_t = bcast(don_b[:], "onb")
              hf = [layer_norm(h[tt], ong_t, onb_t) for tt in range(NT)]
              with tc.tile_pool(name="ps_th", bufs=4, space="PSUM") as psth:
                  hTf = transpose_to_fmajor(hf, psth)
              if True:
                  with tc.tile_pool(name="ps_h1", bufs=2, space="PSUM") as psh1, \
                       tc.tile_pool(name="ps_h2", bufs=1, space="PSUM") as psh2:
                      a1h = []
                      for mc in range(2):
                          ps = psh1.tile([128, T], F32, name="h1ps")
                          for kc in range(NC):
                              for hf_ in range(2):
                                  nc.tensor.matmul(ps[:, hf_ * 512:(hf_ + 1) * 512],
                                                   p1w[:, kc, mc * 128:(mc + 1) * 128],
                                                   hTf[kc][:, hf_ * 512:(hf_ + 1) * 512],
                                                   start=(kc == 0), stop=(kc == NC - 1))
                          t = a1p.tile([128, T], BF16, tag="a1", name="a1h")
                          nc.scalar.activation(t[:], ps[:], AF.Relu, bias=p1b[:, mc:mc + 1])
                          a1h.append(t)
                      ops = psh2.tile([1, T], F32, name="h2ps")
                      for mc in range(2):
                          for hf_ in range(2):
                              nc.tensor.matmul(ops[:, hf_ * 512:(hf_ + 1) * 512],
                                               p2w[:, mc:mc + 1],
                                               a1h[mc][:, hf_ * 512:(hf_ + 1) * 512],
                                               start=(mc == 0), stop=(mc == 1))
                      yt = cst([1, T], F32, "yt")
                      nc.vector.tensor_scalar_add(yt[:], ops[:], p2b[:])
                      nc.sync.dma_start(out=dy[:].rearrange("(a t) -> a t", a=1), in_=yt[:])

    nc.compile()
    return nc


from jax.sharding import Mesh, PartitionSpec, NamedSharding
from jax.experimental.shard_map import shard_map

from concourse import bass2jax
from concourse.bass2jax import _bass_exec_p, install_neuronx_cc_hook


def make_runner(nc, n_cores):
    install_neuronx_cc_hook()
    partition_name = nc.partition_id_tensor.name if nc.partition_id_tensor else None
    in_names, out_names, out_avals, zero_outs = [], [], [], []
    for alloc in nc.m.functions[0].allocations:
        if not isinstance(alloc, mybir.MemoryLocationSet):
            continue
        name = alloc.memorylocations[0].name
        if alloc.kind == "ExternalInput":
            if name != partition_name:
                in_names.append(name)
        elif alloc.kind == "ExternalOutput":
            out_names.append(name)
            shape = tuple(alloc.tensor_shape)
            dtype = mybir.dt.np(alloc.dtype)
            out_avals.append(jax.core.ShapedArray(shape, dtype))
            zero_outs.append(np.zeros(shape, dtype))
    n_params = len(in_names)
    n_outs = len(out_avals)
    all_in_names = list(in_names) + list(out_names)
    if partition_name is not None:
        all_in_names.append(partition_name)

    def _body(*args):
        operands = list(args)
        if partition_name is not None:
            operands.append(bass2jax.partition_id_tensor())
        outs = _bass_exec_p.bind(
            *operands,
            out_avals=tuple(out_avals),
            in_names=tuple(all_in_names),
            out_names=tuple(out_names),
            lowering_input_output_aliases=(),
            sim_require_finite=True,
            sim_require_nnan=True,
            nc=nc,
        )
        return tuple(outs)

    devices = jax.devices()[:n_cores]
    mesh = Mesh(np.asarray(devices), ("core",))
    in_specs = (PartitionSpec("core"),) * (n_params + n_outs)
    out_specs = (PartitionSpec("core"),) * n_outs
    # no donation so the function can be re-invoked with the same buffers
    fn = jax.jit(shard_map(_body, mesh=mesh, in_specs=in_specs,
                           out_specs=out_specs, check_rep=False))

    def run(in_maps):
        per_core = [[np.asarray(m[nm]) for nm in in_names] for m in in_maps]
        concat_in = [np.concatenate([per_core[c][i] for c in range(n_cores)], axis=0)
                     for i in range(n_params)]
        concat_zeros = [np.zeros((n_cores * z.shape[0], *z.shape[1:]), z.dtype)
                        for z in zero_outs]
        sh = NamedSharding(mesh, PartitionSpec("core"))
        args = [jax.device_put(a, sh) for a in concat_in + concat_zeros]

        def call():
            outs = fn(*args)
            jax.block_until_ready(outs)
            return outs

        out_arrs = call()
        results = [
            {nm: np.asarray(out_arrs[i]).reshape(n_cores, *out_avals[i].shape)[c]
             for i, nm in enumerate(out_names)}
            for c in range(n_cores)
        ]
        return results, call

    return run


_CACHE = {}


def kernel(**inputs):
    shared, xs = prep_host(inputs)   # also sets _FLAGS for build_nc
    key = ("k", _FLAGS["trivial_ln"], _FLAGS["trivial_bias"])
    if key not in _CACHE:
        nc = build_nc(num_cores=8)
        run = make_runner(nc, 8)
        _CACHE[key] = run
    run = _CACHE[key]
    in_maps = []
    for c in range(8):
        m = dict(shared)
        m["xT"] = xs[c]
        in_maps.append(m)
    results, _call = run(in_maps)
    ys = [results[c]["y"].reshape(BPC, S, 1) for c in range(8)]
    return np.concatenate(ys, axis=0).astype(np.float32)

